# revision 5
# baseline (speedup 1.0000x reference)
"""GCN VGAE encoder (2-layer, mu/logstd heads) on 8 Trainium2 NeuronCores. v2

Differences from v1:
  - Pair-packed bf16 gather tables: each 256B element = 2 consecutive
    (permuted-row-space) nodes, 64 bf16 each.  Sub-table count drops 4 -> 2
    (pair idx fits int16), cutting chunk padding ~13%.
  - Pre-projection: y2' = disv * (relu(h) @ [W_mu|W_ls]) is aggregated in
    layer 2 (32-wide), so the AllGather ships 64B/node bf16 (1.6MB/core)
    and the heads need no per-block matmuls.
  - Swapped aggregation orientation: matmul(lhsT=one-hot[e,dst],
    rhs=gathered[e,feat]) accumulates [dst,feat] node-major in PSUM; the
    self-loop becomes a DVE add, disv scaling a per-partition activation.
  - Even/odd selection via host-precomputed masked dst-lane arrays
    (dstl_even/dstl_odd; 128 = no-op lane).
"""
import math

import numpy as np
import ml_dtypes

import concourse.bass as bass
import concourse.bacc as bacc
import concourse.mybir as mybir
import concourse.tile as tile
from concourse.bass_utils import run_bass_kernel_spmd
from concourse.masks import make_identity

P = 128
N_CORES = 8
NH = 2                      # sub-table halves (pair idx fits int16)
F32 = mybir.dt.float32
BF16 = mybir.dt.bfloat16
I32 = mybir.dt.int32
I16 = mybir.dt.int16
BF = ml_dtypes.bfloat16
SIM_MODE = False   # sim needs memsets for uninit-read strictness

_CACHE = {}


# ---------------------------------------------------------------- host prep
def _prep(x, edge_index):
    N = x.shape[0]
    in_ch = x.shape[1]
    nbc = math.ceil(math.ceil(N / N_CORES) / P)      # blocks per core (98)
    npc = nbc * P                                    # nodes per core (12544)
    npad = N_CORES * npc                             # padded nodes (100352)
    nblk = N_CORES * nbc                             # blocks (784)
    nsub = npad // NH                                # rows per half (50176)
    npair = nsub // 2                                # pair rows (25088)

    src = np.asarray(edge_index[0]).astype(np.int64)
    dst = np.asarray(edge_index[1]).astype(np.int64)
    E = src.shape[0]

    deg = np.bincount(dst, minlength=npad).astype(np.int32)

    cs = src // npc
    h_e = cs // 4                                     # sub-table half of src

    # sort edges by (dst block, half)
    order = np.argsort((dst >> 7) * NH + h_e, kind="stable")
    dst_s = dst[order]
    src_s = src[order]
    h_s = h_e[order]

    cd = dst_s // npc
    j_s = (dst_s - cd * npc) >> 7                     # block pos within core
    run = (cd * nbc + j_s) * NH + h_s
    nrun = nblk * NH
    counts = np.bincount(run, minlength=nrun)
    rstart = np.zeros(nrun + 1, np.int64)
    np.cumsum(counts, out=rstart[1:])

    cnt = counts.reshape(N_CORES, nbc, NH)
    # Greedy cross-core alignment: core 0 orders its blocks by size; every
    # other core matches each position with its unassigned block minimizing
    # the incremental padded-chunk count max(ceil/128) per half.
    perm = np.empty((N_CORES, nbc), np.int64)
    perm[0] = np.argsort(-cnt[0].sum(axis=1))
    ref_ceil = np.ceil(cnt[0][perm[0]] / P)                    # [nbc, NH]
    for c in range(1, N_CORES):
        ceil_c = np.ceil(cnt[c] / P)                           # [nbc, NH]
        remaining = np.ones(nbc, bool)
        for j in range(nbc):
            cost = np.maximum(ceil_c, ref_ceil[j]).sum(axis=1)
            cost = np.where(remaining, cost, np.inf)
            b = int(np.argmin(cost - 1e-6 * cnt[c].sum(axis=1)))
            perm[c, j] = b
            remaining[b] = False
    pos = np.empty_like(perm)
    for c in range(N_CORES):
        pos[c, perm[c]] = np.arange(nbc)
    cntp = np.take_along_axis(cnt, perm[:, :, None], axis=1)
    S = np.ceil(cntp / P).astype(np.int64).max(axis=0)         # [nbc, NH]
    assert (S > 0).all()
    Tj = S.sum(axis=1)
    off = np.zeros(nbc + 1, np.int64)
    np.cumsum(Tj, out=off[1:])
    T = int(off[-1])
    goff = np.zeros((nbc, NH), np.int64)
    goff[:, 1:] = np.cumsum(S[:, :-1], axis=1)
    Tg = S.sum(axis=0)                                         # chunks per half
    cidx0 = np.zeros((nbc, NH), np.int64)
    cidx0[1:, :] = np.cumsum(S[:-1, :], axis=0)

    posg = pos.reshape(-1)
    rems = src_s - cs[order] * npc
    rows = cs[order] * npc + (rems & 127) * nbc + posg[src_s >> 7]
    rloc = rows - h_s * nsub
    pair = (rloc >> 1).astype(np.int16)
    par = (rloc & 1).astype(np.int64)

    rng = np.random.default_rng(7)
    dstl_pc = np.full((N_CORES, P, T), 128.0, BF)
    idxw = [np.zeros((N_CORES, P, int(Tg[h]) * 8), np.int16) for h in range(NH)]

    for c in range(N_CORES):
        idx_h = [rng.integers(0, npair, (P, int(Tg[h]))).astype(np.int16)
                 for h in range(NH)]               # random pad rows
        for j in range(nbc):
            for h in range(NH):
                r = (c * nbc + int(perm[c, j])) * NH + h
                e0, e1 = rstart[r], rstart[r + 1]
                n_e = e1 - e0
                if n_e == 0:
                    continue
                i = np.arange(n_e)
                lane = i & 127
                col = off[j] + goff[j, h] + (i >> 7)
                lv = ((dst_s[e0:e1] & 127) + 129 * par[e0:e1]).astype(BF)
                dstl_pc[c, lane, col] = lv
                ccol = cidx0[j, h] + (i >> 7)
                idx_h[h][lane, ccol] = pair[e0:e1]
        for h in range(NH):
            flat = idx_h[h].T.ravel()
            w16 = flat.reshape(-1, 16).T
            idxw[h][c] = np.tile(w16, (8, 1))

    gidx = np.concatenate([c * nbc + perm[c] for c in range(N_CORES)])
    xpad = np.zeros((npad, in_ch), np.float32)
    xpad[:N] = np.asarray(x, np.float32)
    x_g = xpad.reshape(nblk, P, in_ch)[gidx].transpose(1, 0, 2).reshape(P, -1)
    deg_g = deg.reshape(nblk, P)[gidx].T.copy()
    x_own = np.stack([
        x_g[:, c * nbc * in_ch:(c + 1) * nbc * in_ch] for c in range(N_CORES)])
    deg_own = np.stack([deg_g[:, c * nbc:(c + 1) * nbc] for c in range(N_CORES)])

    tjmax = int((off[1:] - off[:-1]).max())
    iota = np.tile(np.concatenate([np.arange(P), np.arange(P) + 129.0])
                   .astype(BF), (P, tjmax))

    meta = dict(N=N, E=E, in_ch=in_ch, nbc=nbc, npc=npc, npad=npad,
                nblk=nblk, nsub=nsub, npair=npair, T=T, tjmax=tjmax,
                S=tuple(map(tuple, S.tolist())), perm=perm,
                off=off, goff=goff, cidx0=cidx0,
                Tg=tuple(int(t) for t in Tg))
    arrays = dict(dstl_pc=dstl_pc, idxw=idxw, x_g=x_g,
                  deg_g=deg_g, x_own=x_own, deg_own=deg_own, iota=iota)
    return meta, arrays


# ---------------------------------------------------------------- device build
def _build(meta, in_ch, hid, out_ch):
    nbc, nblk, T = meta["nbc"], meta["nblk"], meta["T"]
    npc, nsub, npair = meta["npc"], meta["nsub"], meta["npair"]
    S, off, goff, cidx0, Tg = (meta["S"], meta["off"], meta["goff"],
                               meta["cidx0"], meta["Tg"])
    tjmax = meta["tjmax"]
    oc2 = 2 * out_ch            # 32: mu|ls concat
    SPC = 14                    # chunks per gather instruction

    nc = bacc.Bacc("TRN2", target_bir_lowering=False, debug=False,
                   num_devices=N_CORES, num_swdge_queues=4)

    x_g_d = nc.dram_tensor("x_g", [P, nblk * in_ch], F32, kind="ExternalInput")
    deg_g_d = nc.dram_tensor("deg_g", [P, nblk], I32, kind="ExternalInput")
    x_o_d = nc.dram_tensor("x_own", [P, nbc * in_ch], F32, kind="ExternalInput")
    deg_o_d = nc.dram_tensor("deg_own", [P, nbc], I32, kind="ExternalInput")
    dstl_d = nc.dram_tensor("dstl_pc", [P, T], BF16, kind="ExternalInput")
    idxw_d = [nc.dram_tensor(f"idxw{h}", [P, Tg[h] * 8], I16,
                             kind="ExternalInput") for h in range(NH)]
    iota_d = nc.dram_tensor("iota", [P, tjmax * 2 * P], BF16, kind="ExternalInput")
    w1_d = nc.dram_tensor("w1", [in_ch, hid], F32, kind="ExternalInput")
    b1c_d = nc.dram_tensor("b1c", [hid, 1], F32, kind="ExternalInput")
    wcat_d = nc.dram_tensor("wcat", [hid, oc2], BF16, kind="ExternalInput")
    bias_d = nc.dram_tensor("bias_cat", [P, oc2], F32, kind="ExternalInput")
    mu_o = nc.dram_tensor("mu_o", [P, nbc * out_ch], F32, kind="ExternalOutput")
    ls_o = nc.dram_tensor("ls_o", [P, nbc * out_ch], F32, kind="ExternalOutput")

    y1tab = [nc.dram_tensor(f"y1tab{h}", [nsub, hid], BF16, kind="Internal")
             for h in range(NH)]
    cc_in = nc.dram_tensor("cc_in", [npc, hid], BF16, kind="Internal")
    cc_out = nc.dram_tensor("cc_out", [N_CORES * npc, hid], BF16,
                            kind="Internal", addr_space="Shared")

    spans = []
    for h in range(NH):
        sp = []
        for c0 in range(0, Tg[h], SPC):
            sp.append((c0, min(SPC, Tg[h] - c0)))
        spans.append(sp)
    # issue order interleaving halves; queue cycles 0..3 in issue order
    order1 = [(h, sp) for i in range(max(len(s) for s in spans))
              for h in range(NH) for sp in spans[h][i:i + 1]]

    with tile.TileContext(nc) as tc:
        with tc.tile_pool(name="const", bufs=1) as cp:
            dstl_t = cp.tile([P, T], BF16)
            idxw_t = [cp.tile([P, Tg[h] * 8], I16, name=f"idxw{h}_t")
                      for h in range(NH)]
            iota_t = cp.tile([P, tjmax * 2 * P], BF16)
            x_gt = cp.tile([P, nblk * in_ch], F32)
            deg_gt = cp.tile([P, nblk], I32)
            x_ot = cp.tile([P, nbc * in_ch], F32)
            deg_ot = cp.tile([P, nbc], I32)
            w1_t = cp.tile([in_ch, hid], F32)
            b1c_t = cp.tile([hid, 1], F32)
            wcat_t = cp.tile([hid, oc2], BF16)
            bias_t = cp.tile([P, oc2], F32)
            ident = cp.tile([P, P], F32)
            disv_g = cp.tile([P, nblk], F32)
            disv_o = cp.tile([P, nbc], F32)
            y1_g = cp.tile([P, nblk * in_ch], F32)
            y1_own = cp.tile([P, nbc * in_ch], F32)
            y2a = cp.tile([P, nbc * hid], BF16)       # y2' padded to 64/node
            out_acc = cp.tile([P, nbc * oc2], F32)

            for dt_, tt in ((x_g_d, x_gt), (deg_g_d, deg_gt), (x_o_d, x_ot),
                            (deg_o_d, deg_ot), (dstl_d, dstl_t),
                            (iota_d, iota_t),
                            (w1_d, w1_t), (b1c_d, b1c_t), (wcat_d, wcat_t),
                            (bias_d, bias_t)):
                nc.sync.dma_start(out=tt[:], in_=dt_[:])
            for h in range(NH):
                nc.sync.dma_start(out=idxw_t[h][:], in_=idxw_d[h][:])
            make_identity(nc, ident[:])
            if SIM_MODE:
                nc.vector.memset(y2a[:], 0.0)

            for deg_t, dv in ((deg_gt, disv_g), (deg_ot, disv_o)):
                nc.vector.tensor_copy(out=dv[:], in_=deg_t[:])
                nc.scalar.activation(dv[:], dv[:],
                                     mybir.ActivationFunctionType.Sqrt,
                                     bias=1.0)
                nc.vector.reciprocal(out=dv[:], in_=dv[:])

            for xt, dv, yt, n in ((x_gt, disv_g, y1_g, nblk),
                                  (x_ot, disv_o, y1_own, nbc)):
                nc.vector.tensor_tensor(
                    out=yt[:].rearrange("p (b c) -> p b c", c=in_ch),
                    in0=xt[:].rearrange("p (b c) -> p b c", c=in_ch),
                    in1=dv[:].rearrange("p (b o) -> p b o", o=1)
                        .to_broadcast([P, n, in_ch]),
                    op=mybir.AluOpType.mult)

            # write y1 fat table rows (bf16, 64 wide, only cols 0:2 used)
            with tc.tile_pool(name="stage", bufs=2) as stp:
                for c in range(N_CORES):
                    st = stp.tile([P, nbc * hid], BF16, tag="st")
                    if SIM_MODE:
                        nc.vector.memset(st[:], 0.0)
                    nc.vector.tensor_copy(
                        out=st[:].rearrange("p (b c) -> p b c", c=hid)
                            [:, :, :in_ch],
                        in_=y1_g[:].rearrange("p (b c) -> p b c", c=in_ch)
                            [:, c * nbc:(c + 1) * nbc, :])
                    nc.scalar.dma_start(
                        out=y1tab[c // 4].ap().rearrange(
                            "(m p b) w -> m p (b w)", m=4, p=P)[c % 4],
                        in_=st[:])

            qn = [0]

            def gather_layer(pool, tabs, tag):
                gtiles = {h: [] for h in range(NH)}
                for h, (c0, n) in order1:
                    gt = pool.tile([P, n * P], BF16, tag=tag)
                    nc.gpsimd.dma_gather(
                        out_ap=gt[:].rearrange("p (n c) -> p n c", c=P),
                        in_ap=tabs[h],
                        idxs_ap=idxw_t[h][:, c0 * 8:(c0 + n) * 8],
                        num_idxs=n * P, num_idxs_reg=n * P,
                        elem_size=P, single_packet=False,
                        queue_num=qn[0] % 4)
                    qn[0] += 1
                    gtiles[h].append((c0, n, gt))
                return gtiles

            def msg(gtiles, h, cidx):
                for c0, n, gt in gtiles[h]:
                    if c0 <= cidx < c0 + n:
                        return gt, cidx - c0
                raise AssertionError

            def build_oh(ohp, xpp, j, tag):
                tj = int(off[j + 1] - off[j])
                c0 = int(off[j])
                de = xpp.tile([P, tj * 2 * P], BF16, tag=tag + "x")
                nc.scalar.activation(
                    de[:].rearrange("p (s f) -> p s f", f=2 * P),
                    dstl_t[:, c0:c0 + tj]
                        .rearrange("p (s o) -> p s o", o=1)
                        .to_broadcast([P, tj, 2 * P]),
                    mybir.ActivationFunctionType.Copy)
                oh = ohp.tile([P, tj * 2 * P], BF16, tag=tag)
                nc.vector.tensor_tensor(
                    out=oh[:], in0=iota_t[:, :tj * 2 * P], in1=de[:],
                    op=mybir.AluOpType.is_equal)
                return oh

            # ---------------- Layer 1 ----------------
            with (
                tc.tile_pool(name="g1", bufs=12) as g1p,
                tc.tile_pool(name="oh1", bufs=4) as ohp,
                tc.tile_pool(name="xp1", bufs=3) as xpp,
                tc.tile_pool(name="sb1", bufs=4) as sbp,
                tc.tile_pool(name="ps1", bufs=2, space="PSUM") as pp1,
                tc.tile_pool(name="pst", bufs=2, space="PSUM") as ppt,
                tc.tile_pool(name="psh", bufs=2, space="PSUM") as pph,
                tc.tile_pool(name="psy", bufs=2, space="PSUM") as ppy,
            ):
                g1 = gather_layer(g1p, [
                    y1tab[h].ap().rearrange("(q two) w -> q (two w)", two=2)
                    for h in range(NH)], "g1")

                for j in range(nbc):
                    oh2 = build_oh(ohp, xpp, j, "oh1")
                    ps1 = pp1.tile([P, in_ch], F32, tag="ps1")
                    first = True
                    for h in range(NH):
                        for t in range(S[j][h]):
                            gt, k = msg(g1, h, int(cidx0[j][h]) + t)
                            col = int(goff[j][h]) + t
                            last = (h == NH - 1) and (t == S[j][NH - 1] - 1)
                            nc.tensor.matmul(
                                out=ps1[:],
                                lhsT=oh2[:, col * 2 * P:col * 2 * P + P],
                                rhs=gt[:, k * P:k * P + in_ch],
                                start=first, stop=False)
                            first = False
                            nc.tensor.matmul(
                                out=ps1[:],
                                lhsT=oh2[:, col * 2 * P + P:(col + 1) * 2 * P],
                                rhs=gt[:, k * P + hid:k * P + hid + in_ch],
                                start=False, stop=last)
                    # self-loop + to SBUF
                    s1sb = sbp.tile([P, in_ch], F32, tag="s1sb")
                    nc.vector.tensor_tensor(
                        out=s1sb[:], in0=ps1[:],
                        in1=y1_own[:, j * in_ch:(j + 1) * in_ch],
                        op=mybir.AluOpType.add)
                    nc.scalar.activation(
                        s1sb[:], s1sb[:],
                        mybir.ActivationFunctionType.Copy,
                        scale=disv_o[:, j:j + 1])
                    psT = ppt.tile([in_ch, P], F32, tag="psT")
                    nc.tensor.matmul(out=psT[:], lhsT=s1sb[:], rhs=ident[:],
                                     is_transpose=True, start=True, stop=True)
                    s1T = sbp.tile([in_ch, P], F32, tag="s1T")
                    nc.vector.tensor_copy(out=s1T[:], in_=psT[:])
                    hTp = pph.tile([hid, P], F32, tag="hTp")
                    nc.tensor.matmul(out=hTp[:], lhsT=w1_t[:], rhs=s1T[:],
                                     start=True, stop=True)
                    hT = sbp.tile([hid, P], BF16, tag="hT")
                    nc.scalar.activation(hT[:], hTp[:],
                                         mybir.ActivationFunctionType.Relu,
                                         bias=b1c_t[:])
                    y2p = ppy.tile([P, oc2], F32, tag="y2p")
                    nc.tensor.matmul(out=y2p[:], lhsT=hT[:], rhs=wcat_t[:],
                                     start=True, stop=True)
                    nc.scalar.activation(
                        y2a[:, j * hid:j * hid + oc2], y2p[:],
                        mybir.ActivationFunctionType.Copy,
                        scale=disv_o[:, j:j + 1])

            half = (nbc // 2) * hid
            cc_v = cc_in.ap().rearrange("(p b) c -> p (b c)", p=P)
            nc.sync.dma_start(out=cc_v[:, :half], in_=y2a[:, :half])
            nc.sync.dma_start(out=cc_v[:, half:], in_=y2a[:, half:])
            nc.gpsimd.collective_compute(
                "AllGather", mybir.AluOpType.bypass,
                ins=[cc_in[:]], outs=[cc_out[:]],
                replica_groups=[list(range(N_CORES))])

            # ---------------- Layer 2 ----------------
            with (
                tc.tile_pool(name="g2", bufs=12) as g2p,
                tc.tile_pool(name="oh2", bufs=4) as ohp2,
                tc.tile_pool(name="xp2", bufs=3) as xpp2,
                tc.tile_pool(name="sb2", bufs=4) as sbp2,
                tc.tile_pool(name="ps2", bufs=6, space="PSUM") as pp2,
            ):
                g2 = gather_layer(g2p, [
                    cc_out.ap()[h * nsub:(h + 1) * nsub, :]
                    .rearrange("(q two) w -> q (two w)", two=2)
                    for h in range(NH)], "g2")

                for j in range(nbc):
                    oh2 = build_oh(ohp2, xpp2, j, "oh2")
                    ps2 = pp2.tile([P, oc2], F32, tag="ps2")
                    first = True
                    for h in range(NH):
                        for t in range(S[j][h]):
                            gt, k = msg(g2, h, int(cidx0[j][h]) + t)
                            col = int(goff[j][h]) + t
                            last = (h == NH - 1) and (t == S[j][NH - 1] - 1)
                            nc.tensor.matmul(
                                out=ps2[:],
                                lhsT=oh2[:, col * 2 * P:col * 2 * P + P],
                                rhs=gt[:, k * P:k * P + oc2],
                                start=first, stop=False)
                            first = False
                            nc.tensor.matmul(
                                out=ps2[:],
                                lhsT=oh2[:, col * 2 * P + P:(col + 1) * 2 * P],
                                rhs=gt[:, k * P + hid:k * P + hid + oc2],
                                start=False, stop=last)
                    fs = sbp2.tile([P, oc2], F32, tag="fs")
                    nc.vector.tensor_tensor(
                        out=fs[:], in0=ps2[:],
                        in1=y2a[:, j * hid:j * hid + oc2],
                        op=mybir.AluOpType.add)
                    nc.scalar.activation(
                        out_acc[:, j * oc2:(j + 1) * oc2], fs[:],
                        mybir.ActivationFunctionType.Copy,
                        scale=disv_o[:, j:j + 1])

            nc.vector.tensor_tensor(
                out=out_acc[:].rearrange("p (b c) -> p b c", c=oc2),
                in0=out_acc[:].rearrange("p (b c) -> p b c", c=oc2),
                in1=bias_t[:].rearrange("p (o c) -> p o c", o=1)
                    .to_broadcast([P, nbc, oc2]),
                op=mybir.AluOpType.add)
            nc.sync.dma_start(
                out=mu_o[:].rearrange("p (b c) -> p b c", c=out_ch),
                in_=out_acc[:].rearrange("p (b c) -> p b c", c=oc2)
                    [:, :, :out_ch])
            nc.sync.dma_start(
                out=ls_o[:].rearrange("p (b c) -> p b c", c=out_ch),
                in_=out_acc[:].rearrange("p (b c) -> p b c", c=oc2)
                    [:, :, out_ch:])

    nc.compile()
    return nc


# ---------------------------------------------------------------- entry point
def kernel(x, edge_index, W1, b1, W_mu, b_mu, W_logstd, b_logstd,
           _want_results=False, _run_kwargs=None):
    x = np.asarray(x, np.float32)
    in_ch, hid, out_ch = W1.shape[0], W1.shape[1], W_mu.shape[1]
    meta, arrays = _prep(x, edge_index)

    key = (meta["N"], meta["E"], in_ch, hid, out_ch, meta["S"])
    if key not in _CACHE:
        _CACHE[key] = _build(meta, in_ch, hid, out_ch)
    nc = _CACHE[key]

    wcat = np.concatenate([np.asarray(W_mu, np.float32),
                           np.asarray(W_logstd, np.float32)], axis=1)
    bias_cat = np.tile(np.concatenate([np.asarray(b_mu, np.float32),
                                       np.asarray(b_logstd, np.float32)]),
                       (P, 1))

    in_maps = []
    for c in range(N_CORES):
        m = dict(
            x_g=arrays["x_g"], deg_g=arrays["deg_g"],
            x_own=arrays["x_own"][c], deg_own=arrays["deg_own"][c],
            dstl_pc=arrays["dstl_pc"][c], iota=arrays["iota"],
            w1=np.asarray(W1, np.float32),
            b1c=np.asarray(b1, np.float32).reshape(-1, 1),
            wcat=wcat.astype(BF), bias_cat=bias_cat)
        for h in range(NH):
            m[f"idxw{h}"] = arrays["idxw"][h][c]
        in_maps.append(m)

    res = run_bass_kernel_spmd(nc, in_maps, core_ids=list(range(N_CORES)),
                               **(_run_kwargs or {}))

    N, nbc, npc = meta["N"], meta["nbc"], meta["npc"]
    out_ch_ = out_ch
    perm = meta["perm"]
    mu = np.empty((N_CORES * npc, out_ch_), np.float32)
    ls = np.empty((N_CORES * npc, out_ch_), np.float32)
    for c in range(N_CORES):
        mo = res.results[c]["mu_o"].reshape(P, nbc, out_ch_).transpose(1, 0, 2)
        lo = res.results[c]["ls_o"].reshape(P, nbc, out_ch_).transpose(1, 0, 2)
        blk = c * npc + perm[c] * P
        for j in range(nbc):
            mu[blk[j]:blk[j] + P] = mo[j]
            ls[blk[j]:blk[j] + P] = lo[j]
    out = (mu[:N], ls[:N])
    if _want_results:
        return out, res
    return out
